# revision 1
# baseline (speedup 1.0000x reference)
"""Trainium2 Bass kernel for decomposed relative-position attention (MViT style).

Reference computation (per batch b, head n):
    score = Q K^T / 8  + qterm_h + qterm_w + kterm_h + kterm_w   (L=1024=32x32, C=64)
    out   = softmax(score) V + Q

All four rel-pos bias terms are absorbed into an augmented QK^T matmul:
    Qaug = [Q/8 ; qterm_h^T ; qterm_w^T]          (128 contraction rows)
    Kaug = [K   ; onehot_h(k) ; onehot_w(k)]
    plus a rank-64 second pass  kterm^T x onehot(q)
The score matrix is computed transposed (S^T[k, q]) so that:
  - exp(S^T) tiles are already in lhsT layout for the PV matmul,
  - the softmax denominator comes free as a ones-column appended to V,
  - normalization uses exp(-ln(denom)) broadcast via a K=1 ones-matmul.

Big matmuls run as float32r (full PE rate); the small rel-pos table matmuls
run in bf16 (their magnitude is ~0.1 so the absolute score error is ~1e-3).

Sharding: head-parallel across the 8 NeuronCores (4 batches x 1 head each).
"""

import os
import sys

import numpy as np

if "/opt/trn_rl_repo" not in sys.path:
    sys.path.insert(0, "/opt/trn_rl_repo")

B, NH, L, C = 4, 8, 1024, 64
NCORES = 8

_CACHED = {}


def _build_nc():
    import concourse.bass as bass
    import concourse.tile as tile
    from concourse import bacc, mybir

    f32 = mybir.dt.float32
    f32r = mybir.dt.float32r
    bf16 = mybir.dt.bfloat16
    Exp = mybir.ActivationFunctionType.Exp
    Ln = mybir.ActivationFunctionType.Ln
    Copy = mybir.ActivationFunctionType.Copy
    mult = mybir.AluOpType.mult
    add = mybir.AluOpType.add

    nc = bacc.Bacc("TRN2", target_bir_lowering=False, debug=False)

    import json
    _grp = os.environ.get("KERNEL_GROUPS", "2,2")
    GRP_SIZE = [int(x) for x in _grp.split(",")]
    assert sum(GRP_SIZE) == B
    GRP_START = []
    _acc = 0
    GRP_OF = {}
    for gi, gsz in enumerate(GRP_SIZE):
        GRP_START.append(_acc)
        for pp in range(_acc, _acc + gsz):
            GRP_OF[pp] = gi
        _acc += gsz
    GRP_START_SET = set(GRP_START)

    qts = nc.dram_tensor("qts", [C, B, L], f32, kind="ExternalInput")
    kts = nc.dram_tensor("kts", [C, B, L], f32, kind="ExternalInput")
    qtb = nc.dram_tensor("qtb", [C, B, L], bf16, kind="ExternalInput")
    ktb = nc.dram_tensor("ktb", [C, B, L], bf16, kind="ExternalInput")
    vaug = nc.dram_tensor("vaug", [B, 128, 8, 65], f32, kind="ExternalInput")
    oh_d = nc.dram_tensor("oh", [64, L], f32, kind="ExternalInput")
    tqh_d = nc.dram_tensor("tqh", [64, 63], bf16, kind="ExternalInput")
    tqw_d = nc.dram_tensor("tqw", [64, 63], bf16, kind="ExternalInput")
    tkh_d = nc.dram_tensor("tkh", [64, 63], bf16, kind="ExternalInput")
    tkw_d = nc.dram_tensor("tkw", [64, 63], bf16, kind="ExternalInput")
    ones_d = nc.dram_tensor("ones1", [1, 64], f32, kind="ExternalInput")
    outt = nc.dram_tensor("outt", [B, 64, L], f32, kind="ExternalOutput")
    DEBUG = bool(int(os.environ.get("KERNEL_DEBUG", "0")))
    if DEBUG:
        dbg_s = nc.dram_tensor("dbg_s", [128, L], f32, kind="ExternalOutput")
        dbg_e = nc.dram_tensor("dbg_e", [128, L], f32, kind="ExternalOutput")
        dbg_unn = nc.dram_tensor("dbg_unn", [65, L], f32, kind="ExternalOutput")
        dbg_re = nc.dram_tensor("dbg_re", [64, 512], f32, kind="ExternalOutput")
        dbg_qa = nc.dram_tensor("dbg_qa", [128, L], f32, kind="ExternalOutput")
        dbg_k2 = nc.dram_tensor("dbg_k2", [64, L], f32, kind="ExternalOutput")
        dbg_r1 = nc.dram_tensor("dbg_r1", [1, 512], f32, kind="ExternalOutput")

    with tile.TileContext(nc) as tc:
        with (
            tc.tile_pool(name="consts", bufs=1) as consts,
            tc.tile_pool(name="persist", bufs=1) as persist,
            tc.tile_pool(name="work", bufs=3) as work,
            tc.tile_pool(name="vpool", bufs=4) as vpool,
            tc.tile_pool(name="expp", bufs=8) as expp,
            tc.tile_pool(name="unnp", bufs=2) as unnp,
            tc.tile_pool(name="outp", bufs=2) as outp,
        ):
            # ---- constants (aug-phase dependencies first) ----
            tqh_t = consts.tile([64, 63], bf16)
            nc.sync.dma_start(tqh_t, tqh_d[:])
            tqw_t = consts.tile([64, 63], bf16)
            nc.sync.dma_start(tqw_t, tqw_d[:])
            tkh_t = consts.tile([64, 63], bf16)
            nc.sync.dma_start(tkh_t, tkh_d[:])
            tkw_t = consts.tile([64, 63], bf16)
            nc.sync.dma_start(tkw_t, tkw_d[:])

            # ---- persistent per-core tensors (all 4 pairs resident) ----
            QaugT = persist.tile([128, B, L], f32)
            KaugT = persist.tile([128, B, L], f32)
            KT2 = persist.tile([64, B, L], f32)
            qtb_t = persist.tile([C, B, L], bf16)
            ktb_t = persist.tile([C, B, L], bf16)

            nc.gpsimd.dma_start(qtb_t, qtb[:])
            nc.gpsimd.dma_start(ktb_t, ktb[:])

            oh_t = consts.tile([64, L], f32)
            nc.sync.dma_start(oh_t.bitcast(f32r), oh_d[:].bitcast(f32r))
            ones_t = consts.tile([1, 64], f32)
            nc.sync.dma_start(ones_t, ones_d[:])
            for p in range(B):
                nc.sync.dma_start(QaugT[0:64, p, :].bitcast(f32r), qts[:, p, :].bitcast(f32r))
                nc.sync.dma_start(KaugT[0:64, p, :].bitcast(f32r), kts[:, p, :].bitcast(f32r))
            for p in range(B):
                nc.sync.dma_start(KaugT[64:128, p, :].bitcast(f32r), oh_d[:].bitcast(f32r))

            # ---- rel-pos augmentation terms (bf16), batched across pairs ----
            # For each 4-group chunk t: 4 matmuls of [64,32]^T @ [64, 4x32]
            # into one PSUM bank laid out [g, pair, 32], then one copy out.
            with tc.tile_pool(name="ps_aug", bufs=8, space="PSUM") as ps_aug:
                # (table, src bf16 tile, dst tile, dst row base, w_major)
                terms = [
                    (tqh_t, qtb_t, QaugT, 64, False),
                    (tqw_t, qtb_t, QaugT, 96, True),
                    (tkh_t, ktb_t, KT2, 0, False),
                    (tkw_t, ktb_t, KT2, 32, True),
                ]
                for tbl, src, dst, row0, wmaj in terms:
                    if wmaj:
                        # columns g :: 32  (fixed w=g, h varying)
                        src_r = src.rearrange("c p (h g) -> c p g h", g=32)
                    for t in range(8):
                        ps = ps_aug.tile([32, 512], f32, tag="aug", name="augps")
                        for i in range(4):
                            g = 4 * t + i
                            lhsT = tbl[:, 31 - g: 63 - g]
                            if wmaj:
                                rhs = src_r[:, :, g, :]
                            else:
                                rhs = src[:, :, 32 * g: 32 * g + 32]
                            nc.tensor.matmul(
                                ps[:, 128 * i: 128 * (i + 1)], lhsT, rhs,
                                start=True, stop=True,
                            )
                        # psum layout: [j, (i, pair, x)] with x = w or h
                        src_ap = ps.rearrange("j (i p x) -> j p i x", i=4, p=4)
                        if wmaj:
                            # dst cols q = 32*x + (4t + i)
                            dst_ap = dst[row0:row0 + 32].rearrange(
                                "j p (x g) -> j p g x", g=32
                            )[:, :, 4 * t: 4 * t + 4, :]
                        else:
                            # dst cols q = 32*(4t+i) + x : contiguous 128 block
                            dst_ap = dst[row0:row0 + 32, :, 128 * t: 128 * (t + 1)].rearrange(
                                "j p (i x) -> j p i x", x=32
                            )
                        if t % 2 == 0:
                            nc.vector.tensor_copy(dst_ap.bitcast(f32r), src_ap)
                        else:
                            nc.scalar.activation(dst_ap.bitcast(f32r), src_ap, Copy)

            # ---- main attention loop (all big matmuls in float32r) ----
            unns = []
            pvs = []
            with (
                tc.tile_pool(name="ps_s", bufs=3, space="PSUM") as ps_s,
                tc.tile_pool(name="ps_pv", bufs=1, space="PSUM") as ps_pv,
            ):
                for p in range(B):
                    vg = vpool.tile([128, 8, 65], f32, tag="vg", name="vg")
                    nc.sync.dma_start(vg.bitcast(f32r), vaug[p].bitcast(f32r))

                    pv = ps_pv.tile([65, L], f32, tag="pv", name="pv")
                    pvs.append(pv)
                    for kb in range(8):
                        sp = ps_s.tile([128, L], f32, tag="sp", name="sp")
                        kcols = slice(128 * kb, 128 * (kb + 1))
                        for ch in range(2):
                            cs = slice(512 * ch, 512 * (ch + 1))
                            nc.tensor.matmul(
                                sp[:, cs],
                                KaugT[:, p, kcols].bitcast(f32r),
                                QaugT[:, p, cs].bitcast(f32r),
                                start=True, stop=False,
                            )
                        for ch in range(2):
                            cs = slice(512 * ch, 512 * (ch + 1))
                            nc.tensor.matmul(
                                sp[:, cs],
                                KT2[:, p, kcols].bitcast(f32r),
                                oh_t[:, cs].bitcast(f32r),
                                start=False, stop=True,
                            )
                        et = expp.tile([128, L], f32, tag="et", name="et")
                        if DEBUG and p == 0 and kb == 0:
                            sdmp = work.tile([128, L], f32, tag="sdmp", name="sdmp")
                            nc.vector.tensor_copy(sdmp, sp)
                            nc.sync.dma_start(dbg_s[:], sdmp)
                        nc.scalar.activation(et.bitcast(f32r), sp, Exp)
                        if DEBUG and p == 0 and kb == 0:
                            nc.sync.dma_start(dbg_e[:], et)
                        for ch in range(2):
                            cs = slice(512 * ch, 512 * (ch + 1))
                            nc.tensor.matmul(
                                pv[:, cs],
                                vg[:, kb, :].bitcast(f32r),
                                et[:, cs].bitcast(f32r),
                                start=(kb == 0), stop=(kb == 7),
                            )

                    # free the PSUM accumulator early; normalize below.
                    # The last pair's PSUM is never recycled, so it skips the
                    # SBUF copy and is read directly during normalization.
                    if p < B - 1:
                        unn = unnp.tile([65, L], f32, tag="unn", name="unn")
                        nc.vector.tensor_copy(unn, pv)
                        unns.append(unn)
                    else:
                        unns.append(pv)
                    if DEBUG and p == 0:
                        nc.sync.dma_start(dbg_unn[:], unn)
                        nc.sync.dma_start(dbg_qa[:], QaugT[:, 0, :])
                        nc.sync.dma_start(dbg_k2[:], KT2[:, 0, :])

            # ---- normalization via DVE StreamTranspose + exact reciprocal ----
            # dnb[j, q] = denom[q] (gpsimd broadcast); block-transpose puts
            # denom values on partitions; reciprocal runs on a [64, 32] slice
            # (all 1024 values); broadcast-copy + transpose back yields
            # rS[j, q] = 1/denom[q] with no ACT table switches at all.
            for p in range(B):
                unn = unns[p]
                dnr = work.tile([1, L], f32, tag="dnr", name="dnr", bufs=2)
                nc.vector.tensor_copy(dnr, pvs[p][64:65, :])
                dnb = work.tile([64, L], f32, tag="dnb", name="dnb", bufs=2)
                nc.gpsimd.partition_broadcast(dnb, dnr)
                dnT = work.tile([64, L], f32, tag="dnT", name="dnT", bufs=2)
                nc.vector.transpose(dnT, dnb)
                rT = work.tile([64, 32], f32, tag="rT", name="rT", bufs=2)
                nc.vector.reciprocal(
                    rT, dnT.rearrange("i (c j) -> i j c", j=32)[:, 0, :]
                )
                rE = work.tile([64, 32, 32], f32, tag="rE", name="rE", bufs=2)
                nc.vector.tensor_copy(
                    rE, rT[:, :, None].to_broadcast((64, 32, 32))
                )
                rS = work.tile([64, L], f32, tag="rS", name="rS", bufs=2)
                nc.vector.transpose(rS, rE.rearrange("i c j -> i (c j)"))
                if DEBUG and p == 0:
                    nc.sync.dma_start(dbg_r1[:], rS[0:1, 0:512])
                    nc.sync.dma_start(dbg_re[:], rS[:, 0:512])
                ot = outp.tile([64, L], f32, tag="ot", name="ot")
                for ch in range(2):
                    cs = slice(512 * ch, 512 * (ch + 1))
                    tmp = work.tile([64, 512], f32, tag="tmp", name="tmp")
                    nc.vector.tensor_mul(tmp, unn[0:64, cs], rS[:, cs])
                    nc.vector.scalar_tensor_tensor(
                        ot[:, cs], QaugT[0:64, p, cs], 8.0, tmp, mult, add
                    )
                    nc.sync.dma_start(outt[p, :, cs], ot[:, cs])

    nc.compile()
    return nc


def _host_consts():
    oh = np.zeros((64, L), np.float32)
    qq = np.arange(L)
    oh[qq // 32, qq] = 1.0
    oh[32 + qq % 32, qq] = 1.0
    ones1 = np.ones((1, 64), np.float32)
    return oh, ones1


def kernel(query, key_input, value, rel_h_q, rel_w_q, rel_h_k, rel_w_k):
    from concourse.bass_utils import run_bass_kernel_spmd

    query = np.asarray(query, np.float32)
    key_input = np.asarray(key_input, np.float32)
    value = np.asarray(value, np.float32)
    rel_h_q = np.asarray(rel_h_q, np.float32)
    rel_w_q = np.asarray(rel_w_q, np.float32)
    rel_h_k = np.asarray(rel_h_k, np.float32)
    rel_w_k = np.asarray(rel_w_k, np.float32)

    if "nc" not in _CACHED:
        _CACHED["nc"] = _build_nc()
    nc = _CACHED["nc"]

    import ml_dtypes

    bf = ml_dtypes.bfloat16
    oh, ones1 = _host_consts()
    tqh = np.ascontiguousarray(rel_h_q[::-1].T).astype(bf)
    tqw = np.ascontiguousarray(rel_w_q[::-1].T).astype(bf)
    tkh = np.ascontiguousarray(rel_h_k.T).astype(bf)
    tkw = np.ascontiguousarray(rel_w_k.T).astype(bf)

    in_maps = []
    for n in range(NCORES):
        qt = np.ascontiguousarray(query[:, n].transpose(2, 0, 1))
        kt = np.ascontiguousarray(key_input[:, n].transpose(2, 0, 1))
        v = value[:, n]
        va = np.concatenate([v, np.ones((B, L, 1), np.float32)], -1)
        va = np.ascontiguousarray(va.reshape(B, 8, 128, 65).transpose(0, 2, 1, 3))
        in_maps.append(
            dict(qts=qt * 0.125, kts=kt, qtb=qt.astype(bf), ktb=kt.astype(bf),
                 vaug=va, oh=oh, tqh=tqh, tqw=tqw, tkh=tkh, tkw=tkw, ones1=ones1)
        )

    res = run_bass_kernel_spmd(
        nc, in_maps, core_ids=list(range(NCORES)),
        trace=bool(int(os.environ.get("KERNEL_TRACE", "0"))),
    )
    _CACHED["last_result"] = res

    out = np.stack([r["outt"] for r in res.results], axis=1)  # [B, NH, 64, L]
    return np.ascontiguousarray(out.transpose(0, 1, 3, 2)).astype(np.float32)

